# revision 20
# baseline (speedup 1.0000x reference)
"""BiGraphConv (GNN message passing) Trainium2 kernel, 8-core SPMD.

out = x_dst @ W_self.T + b_self + scatter_add_dst(w_e * x_src[src_e]) @ W_nei.T

Formulated aggregate-first, per dst-shard:
    agg[d]  = sum_{e: dst_e=d} w_e * x_src[src_e]     (gather + one-hot matmul)
    out'[d] = W_nei @ agg[d] + W_self @ x_dst[d] + b  (feature-major matmuls)

Sharding: dst nodes partitioned across 8 cores (12500 each); x_src replicated;
edges bucketed by (dst-core, src-chunk, dst) on host. Edge gather + one-hot
aggregation run in bf16 (error ~2e-3); transform + self term in fp32. Output
assembled/transposed on host.
"""
import sys
import inspect
import re
import numpy as np

for _p in ("/opt/trn_rl_repo", "/root/.axon_site/_ro/trn_rl_repo"):
    if _p not in sys.path:
        sys.path.insert(0, _p)

from contextlib import ExitStack

import ml_dtypes
import concourse.bass as bass
import concourse.tile as tile
from concourse import bacc, mybir
from concourse.bass_utils import run_bass_kernel_spmd

# problem constants (hardcoded per task contract)
N_SRC = 100000
N_DST = 100000
E = 1250000
F = 64          # feature dim (in == out == 64)
NC = 8          # cores
SHARD = N_DST // NC          # 12500 dst rows per core
G = 70                       # dst rows per aggregation group
NG = (SHARD + G - 1) // G    # 196 groups per core
NCH = 4                      # src chunks (int16 index limit)
CHROWS = N_SRC // NCH        # 25000 rows per chunk window
W = 32                       # gather window width in 128-edge columns
KB = 16                      # one-hot batch width in columns
DMA_SCRATCH = 16384          # SWDGE ring bytes per partition (default)
TCH = 490                    # transform chunk (dst cols; multiple of G)
NTC = (SHARD + TCH - 1) // TCH   # 25 transform chunks
USE_BF16 = True              # bf16 gather + aggregation (fp32 transform)

P = 128
XPAD = 128                   # padded bf16 row length (256B stride)

_patched_gather = None


def _get_patched_gather(nc):
    """dma_gather with the 256B-payload assert relaxed for non-transpose.

    The ucode's row-stride field is in 256B units (elem_step stays 256B via
    the padded source), but the payload may be 128B; verified on HW.
    """
    global _patched_gather
    if _patched_gather is not None:
        return _patched_gather
    cls = type(nc.gpsimd)
    src = inspect.getsource(cls.dma_gather)
    src = src.replace(
        """        assert (
            elem_size_bytes > 0 and elem_size_bytes % 256 == 0
        )  # transpose restriction""",
        """        assert elem_size_bytes > 0
        if transpose:
            assert elem_size_bytes % 256 == 0""")
    src = re.sub(r"^    def dma_gather", "def dma_gather", src)
    src = re.sub(r"\n    ", "\n", src)
    ns = vars(sys.modules[cls.__module__]).copy()
    exec(compile(src, "<patched_dma_gather>", "exec"), ns)
    _patched_gather = ns["dma_gather"]
    return _patched_gather


def _host_prep(x_src, x_dst, edge_index_sd, edge_weight, W_nei, W_self, b_self):
    src = np.asarray(edge_index_sd[0], dtype=np.int64)
    dst = np.asarray(edge_index_sd[1], dtype=np.int64)
    ew = np.asarray(edge_weight, dtype=np.float32)
    x_dst = np.asarray(x_dst, dtype=np.float32)

    core = dst // SHARD
    chunk = src // CHROWS
    dl = dst % SHARD          # shard-local dst id
    grp = dl // G

    # layout order: (core, chunk, dst) -> per-core chunk-major, dst ascending
    order = np.lexsort((dl, chunk, core))
    core_s = core[order]
    chunk_s = chunk[order]
    dl_s = dl[order]
    grp_s = grp[order]
    src_s = src[order]
    ew_s = ew[order]

    # edge counts per (core, group, chunk)
    key = (core_s * NG + grp_s) * NCH + chunk_s
    cnt = np.bincount(key, minlength=NC * NG * NCH).reshape(NC, NG, NCH)

    # common column layout: per (group, chunk) slot width = max over cores
    cols_gr = np.ceil(cnt / P).astype(np.int64).max(axis=0)  # [NG, NCH]
    empty = cols_gr.sum(axis=1) == 0
    cols_gr[empty, 0] = 1  # every group owns >=1 column (zero contribution)

    # column start of each slot, chunk-major then group order
    cols_rg = cols_gr.T                      # [NCH, NG]
    flat = cols_rg.reshape(-1)
    starts = np.zeros_like(flat)
    np.cumsum(flat[:-1], out=starts[1:])
    col_start_rg = starts.reshape(NCH, NG)   # [NCH, NG] global col index
    cols_r = cols_rg.sum(axis=1)             # columns per region
    base_r = np.zeros(NCH, dtype=np.int64)
    np.cumsum(cols_r[:-1], out=base_r[1:])
    totcols = int(cols_r.sum())
    # padded (KB-aligned) per-region table layout for batched one-hots
    cols_r_pad = ((cols_r + KB - 1) // KB) * KB
    base_r_pad = np.zeros(NCH, dtype=np.int64)
    np.cumsum(cols_r_pad[:-1], out=base_r_pad[1:])
    totcols_pad = int(cols_r_pad.sum())

    ftype = np.float16 if USE_BF16 else np.float32

    # per-core tables
    core_cnt = np.bincount(core_s, minlength=NC)
    core_off = np.zeros(NC + 1, dtype=np.int64)
    np.cumsum(core_cnt, out=core_off[1:])

    per_core = []
    for c in range(NC):
        s, e = core_off[c], core_off[c + 1]
        r_c = chunk_s[s:e]
        g_c = grp_s[s:e]
        dl_c = dl_s[s:e]
        src_c = src_s[s:e]
        ew_c = ew_s[s:e]
        n = e - s
        # position within (group, chunk) run
        sid = r_c * NG + g_c
        run_starts = np.zeros(n, dtype=np.int64)
        if n:
            brk = np.flatnonzero(np.diff(sid)) + 1
            rb = np.r_[0, brk]
            run_starts = np.repeat(rb, np.diff(np.r_[rb, n]))
        pos = np.arange(n, dtype=np.int64) - run_starts
        tgt = col_start_rg[r_c, g_c] * P + pos      # flat slot position

        dstl_flat = np.full(totcols * P, -1.0, dtype=np.float32)
        w_flat = np.zeros(totcols * P, dtype=np.float32)
        idx_flat = np.zeros(totcols * P, dtype=np.int16)
        dstl_flat[tgt] = (dl_c - g_c * G).astype(np.float32)
        w_flat[tgt] = ew_c
        idx_flat[tgt] = (src_c - r_c * CHROWS).astype(np.int16)

        # tables in padded-region layout (each region KB-aligned) for the
        # batched one-hot construction
        dstl_p = np.full(totcols_pad * P, -1.0, dtype=ftype)
        w_p = np.zeros(totcols_pad * P, dtype=ftype)
        for r in range(NCH):
            a0, a1 = base_r[r] * P, (base_r[r] + cols_r[r]) * P
            b0 = base_r_pad[r] * P
            dstl_p[b0:b0 + (a1 - a0)] = dstl_flat[a0:a1].astype(ftype)
            w_p[b0:b0 + (a1 - a0)] = w_flat[a0:a1].astype(ftype)
        dstl_tab = np.ascontiguousarray(dstl_p.reshape(totcols_pad, P).T)
        w_tab = np.ascontiguousarray(w_p.reshape(totcols_pad, P).T)

        # idx16 tables: per region, wrapped [16, cols_r*8] then replicated x8
        idx_parts = []
        for r in range(NCH):
            b0, b1 = base_r[r] * P, (base_r[r] + cols_r[r]) * P
            seg = idx_flat[b0:b1]
            t16 = seg.reshape(-1, 16).T                  # [16, cols_r*8]
            idx_parts.append(np.tile(t16, (8, 1)))       # [128, cols_r*8]
        idx_tab = np.ascontiguousarray(np.concatenate(idx_parts, axis=1))

        xdt = np.ascontiguousarray(
            x_dst[c * SHARD:(c + 1) * SHARD].T.astype(ftype))
        per_core.append({"dstl": dstl_tab, "w": w_tab, "idx16": idx_tab,
                         "xdt": xdt})

    meta = {
        "cols_gr": cols_gr, "col_start_rg": col_start_rg,
        "cols_r": cols_r, "base_r": base_r, "totcols": totcols,
        "cols_r_pad": cols_r_pad, "base_r_pad": base_r_pad,
        "totcols_pad": totcols_pad,
    }
    common = {
        "iota": np.tile(np.repeat(np.arange(G), KB).astype(ftype), (P, 1)),
        "wn": np.ascontiguousarray(np.asarray(W_nei, np.float32).T),
        "ws": np.ascontiguousarray(np.asarray(W_self, np.float32).T
                                   .astype(ftype)),
        "bias": np.asarray(b_self, np.float32).reshape(F, 1),
    }
    return meta, per_core, common


def _build_program(meta):
    cols_gr = meta["cols_gr"]
    col_start_rg = meta["col_start_rg"]
    cols_r = meta["cols_r"]
    base_r = meta["base_r"]
    totcols = meta["totcols"]
    base_r_pad = meta["base_r_pad"]
    cols_r_pad = meta["cols_r_pad"]
    totcols_pad = meta["totcols_pad"]
    totidx = int(cols_r.sum()) * 8

    nc = bacc.Bacc("TRN2", target_bir_lowering=False, debug=False,
                   enable_asserts=False, num_devices=NC,
                   dynamic_dma_scratch_size=DMA_SCRATCH)
    f32 = mybir.dt.float32
    DT = mybir.dt.float16 if USE_BF16 else f32
    xcols = XPAD if USE_BF16 else F
    x_src_t = nc.dram_tensor("x_src", (N_SRC, xcols), DT,
                             kind="ExternalInput")
    xdt_t = nc.dram_tensor("xdt", (F, SHARD), DT, kind="ExternalInput")
    idx_t = nc.dram_tensor("idx16", (P, totidx), mybir.dt.int16,
                           kind="ExternalInput")
    dstl_t = nc.dram_tensor("dstl", (P, totcols_pad), DT,
                            kind="ExternalInput")
    w_t = nc.dram_tensor("w", (P, totcols_pad), DT, kind="ExternalInput")
    iota_t = nc.dram_tensor("iota", (P, G * KB), DT, kind="ExternalInput")
    wn_t = nc.dram_tensor("wn", (F, F), f32, kind="ExternalInput")
    ws_t = nc.dram_tensor("ws", (F, F), DT, kind="ExternalInput")
    bias_t = nc.dram_tensor("bias", (F, 1), f32, kind="ExternalInput")
    out_t = nc.dram_tensor("outT", (F, SHARD), f32, kind="ExternalOutput")

    gather_fn = _get_patched_gather(nc) if USE_BF16 else None

    # per-group pair lists: (region, global col); chain order region-major
    group_pairs = []
    for g in range(NG):
        pairs = []
        for r in range(NCH):
            c0 = col_start_rg[r, g]
            for c in range(c0, c0 + cols_gr[g, r]):
                pairs.append((r, int(c)))
        group_pairs.append(pairs)
    # variable window widths: small ramp-in, W steady, small tail
    def mk_widths(cr):
        widths = []
        rem = int(cr)
        for w0 in (8, 24):
            if rem <= 0:
                break
            take = min(w0, rem)
            widths.append(take)
            rem -= take
        while rem > 48:
            widths.append(W)
            rem -= W
        for w0 in (16, 16, 8, 8):
            if rem <= 0:
                break
            take = min(w0, rem)
            widths.append(take)
            rem -= take
        while rem > 0:
            widths.append(min(8, rem))
            rem -= min(8, rem)
        return widths
    win_widths = [mk_widths(cols_r[r]) for r in range(NCH)]
    win_starts = []
    for r in range(NCH):
        st, acc = [], 0
        for w0 in win_widths[r]:
            st.append(acc)
            acc += w0
        win_starts.append(st)
    n_win = max(len(ws_) for ws_ in win_widths)

    def col_to_win(r, o):
        import bisect
        return bisect.bisect_right(win_starts[r], o) - 1

    gwin = []
    gbat = []
    for g in range(NG):
        wk = 0
        bk = 0
        for (r, c) in group_pairs[g]:
            wk = max(wk, col_to_win(r, c - int(base_r[r])))
            bk = max(bk, (c - base_r[r]) // KB)
        gwin.append(wk)
        gbat.append(bk)

    with tile.TileContext(nc) as tc:
        with ExitStack() as ctx:
            const = ctx.enter_context(tc.tile_pool(name="const", bufs=1))
            msgp = [ctx.enter_context(tc.tile_pool(name=f"msg{r}", bufs=3))
                    for r in range(NCH)]
            megs = ctx.enter_context(tc.tile_pool(name="megs", bufs=3))
            megp = ctx.enter_context(tc.tile_pool(name="mega", bufs=14))
            aggp = ctx.enter_context(tc.tile_pool(name="agg", bufs=3))
            xdtp = ctx.enter_context(tc.tile_pool(name="xdtp", bufs=3))
            outp = ctx.enter_context(tc.tile_pool(name="outp", bufs=3))
            psg = ctx.enter_context(tc.tile_pool(name="psg", bufs=6,
                                                 space="PSUM"))
            pst = ctx.enter_context(tc.tile_pool(name="pst", bufs=2,
                                                 space="PSUM"))

            idx_rs = []
            for r in range(NCH):
                i0 = int(base_r[r]) * 8
                i1 = i0 + int(cols_r[r]) * 8
                idx_r = const.tile([P, i1 - i0], mybir.dt.int16,
                                   tag=f"idx{r}")
                nc.sync.dma_start(idx_r[:], idx_t.ap()[:, i0:i1])
                idx_rs.append(idx_r)
            iota_s = const.tile([P, G * KB], DT)
            nc.sync.dma_start(iota_s[:], iota_t.ap())
            dstl_s = const.tile([P, totcols_pad], DT)
            nc.sync.dma_start(dstl_s[:], dstl_t.ap())
            w_s = const.tile([P, totcols_pad], DT)
            nc.sync.dma_start(w_s[:], w_t.ap())
            wn_s = const.tile([F, F], f32)
            nc.sync.dma_start(wn_s[:], wn_t.ap())
            ws_s = const.tile([F, F], DT)
            nc.sync.dma_start(ws_s[:], ws_t.ap())
            bias_s = const.tile([F, 1], f32)
            nc.sync.dma_start(bias_s[:], bias_t.ap())

            win_tiles = [[None] * n_win for _ in range(NCH)]
            n_bat = [int((cols_r[r] + KB - 1) // KB) for r in range(NCH)]
            bat_tiles = [[None] * max(1, n_bat[r]) for r in range(NCH)]

            def emit_batch(r, bk):
                tb0 = int(base_r_pad[r]) + bk * KB
                eq = megs.tile([P, G * KB], DT, tag="eq")
                nc.vector.tensor_tensor(
                    out=eq[:].rearrange("p (g k) -> p g k", k=KB),
                    in0=iota_s[:].rearrange("p (g k) -> p g k", k=KB),
                    in1=dstl_s[:, tb0:tb0 + KB].unsqueeze(1)
                        .broadcast_to([P, G, KB]),
                    op=mybir.AluOpType.is_equal)
                pm = megp.tile([P, G * KB], DT, tag="pm")
                nc.vector.tensor_tensor(
                    out=pm[:].rearrange("p (g k) -> p g k", k=KB),
                    in0=eq[:].rearrange("p (g k) -> p g k", k=KB),
                    in1=w_s[:, tb0:tb0 + KB].unsqueeze(1)
                        .broadcast_to([P, G, KB]),
                    op=mybir.AluOpType.mult)
                bat_tiles[r][bk] = pm

            def emit_window(wk):
                for r in range(NCH):
                    if wk >= len(win_widths[r]):
                        continue
                    c0 = win_starts[r][wk]
                    wcols = int(win_widths[r][wk])
                    mt = msgp[r].tile([P, W * F], DT, tag=f"m{r}")
                    out3d = mt[:, :wcols * F].rearrange(
                        "p (c f) -> p c f", f=F)
                    i0 = c0 * 8
                    nidx = wcols * P
                    if USE_BF16:
                        gather_fn(
                            nc.gpsimd,
                            out_ap=out3d,
                            in_ap=x_src_t.ap()[r * CHROWS:(r + 1) * CHROWS,
                                               :F],
                            idxs_ap=idx_rs[r][:, i0:i0 + wcols * 8],
                            num_idxs=nidx, num_idxs_reg=nidx, elem_size=F,
                            elem_step=XPAD, single_packet=False)
                    else:
                        nc.gpsimd.dma_gather(
                            out_ap=out3d,
                            in_ap=x_src_t.ap()[r * CHROWS:(r + 1) * CHROWS,
                                               :],
                            idxs_ap=idx_rs[r][:, i0:i0 + wcols * 8],
                            num_idxs=nidx, num_idxs_reg=nidx, elem_size=F,
                            single_packet=False)
                    win_tiles[r][wk] = mt

            emitted = 0
            bat_emitted = 0
            for t in range(NTC):
                csize = min(TCH, SHARD - t * TCH)
                glo = t * (TCH // G)
                ghi = min(NG, glo + (TCH // G))
                agg_tile = aggp.tile([F, TCH], f32, tag="agg")
                for g in range(glo, ghi):
                    while emitted <= gwin[g] and emitted < n_win:
                        emit_window(emitted)
                        emitted += 1
                    while bat_emitted <= gbat[g]:
                        done = True
                        for r in range(NCH):
                            if bat_emitted < n_bat[r]:
                                emit_batch(r, bat_emitted)
                                done = False
                        bat_emitted += 1
                        if done:
                            break
                    gsize = min(G, SHARD - g * G)
                    ps = psg.tile([F, G], f32, tag="ps")
                    pairs = group_pairs[g]
                    for j, (r, c) in enumerate(pairs):
                        o = c - int(base_r[r])
                        lcw = col_to_win(r, o)
                        lc = o - win_starts[r][lcw]
                        mt = win_tiles[r][lcw]
                        pm = bat_tiles[r][o // KB]
                        jk = o % KB
                        rhs = pm[:].rearrange(
                            "p (g k) -> p g k", k=KB)[:, :, jk]
                        nc.tensor.matmul(
                            out=ps[:], lhsT=mt[:, lc * F:(lc + 1) * F],
                            rhs=rhs, start=(j == 0),
                            stop=(j == len(pairs) - 1))
                    off = (g - glo) * G
                    nc.scalar.copy(agg_tile[:, off:off + gsize],
                                   ps[:, :gsize])
                # transform this chunk of 512 dsts
                xdt_s = xdtp.tile([F, TCH], DT, tag="xdt")
                nc.sync.dma_start(xdt_s[:, :csize],
                                  xdt_t.ap()[:, t * TCH:t * TCH + csize])
                ps2 = pst.tile([F, TCH], f32, tag="ps2")
                nc.tensor.matmul(out=ps2[:, :csize], lhsT=wn_s[:],
                                 rhs=agg_tile[:, :csize], start=True,
                                 stop=False)
                nc.tensor.matmul(out=ps2[:, :csize], lhsT=ws_s[:],
                                 rhs=xdt_s[:, :csize], start=False, stop=True)
                osb = outp.tile([F, TCH], f32, tag="osb")
                nc.vector.tensor_scalar(
                    out=osb[:, :csize], in0=ps2[:, :csize],
                    scalar1=bias_s[:, 0:1], scalar2=None,
                    op0=mybir.AluOpType.add)
                nc.sync.dma_start(out_t.ap()[:, t * TCH:t * TCH + csize],
                                  osb[:, :csize])

    nc.compile()
    return nc


def _prep_x_src(x_src):
    x_src = np.asarray(x_src, dtype=np.float32)
    if USE_BF16:
        xp = np.zeros((N_SRC, XPAD), dtype=np.float16)
        xp[:, :F] = x_src.astype(np.float16)
        return xp
    return x_src


def run(inputs, trace=False):
    meta, per_core, common = _host_prep(
        inputs["x_src"], inputs["x_dst"], inputs["edge_index_sd"],
        inputs["edge_weight"], inputs["W_nei"], inputs["W_self"],
        inputs["b_self"])
    nc = _build_program(meta)
    xs = _prep_x_src(inputs["x_src"])
    in_maps = []
    for c in range(NC):
        m = {"x_src": xs}
        m.update(common)
        m.update(per_core[c])
        in_maps.append(m)
    res = run_bass_kernel_spmd(nc, in_maps, core_ids=list(range(NC)),
                               trace=trace)
    out = np.empty((N_DST, F), dtype=np.float32)
    for c in range(NC):
        out[c * SHARD:(c + 1) * SHARD] = res.results[c]["outT"].T
    return out, res


def kernel(**inputs) -> np.ndarray:
    out, _ = run(inputs, trace=False)
    return out
